# revision 21
# baseline (speedup 1.0000x reference)
"""Trainium2 Bass kernel for AttentionDownsampler (nn_AttentionDownsampler_10264971837445).

Math (per batch b):
  patches[b, Y, X, p=(y,xi), c] = hr[b, c, 14Y+y, 14X+xi]
  logits[b, Y, X, p] = sum_c patches * w[c] + ab
  l2 = logits * mask[b, Y, X] * wkk[p] + bkk[p]
  attn = softmax_p(l2)
  out[b, c, Y, X] = sum_p patches[..., p, c] * attn[p]

Sharding: 8 cores = 4 batches x 2 halves of the H(=Y) axis. Per-core shard is
patch-contiguous on the host: [384, 8 rows, 16 X, 196 px], sent as bf16
(weighted-average output keeps ~4e-3 rel err, well under the 2e-2 gate).

Per-core kernel (4 row-PAIR iterations), v2:
  - DMA 3 c-chunk tiles [128, 2 rows, 16 X, 196 px] (bf16)
  - PE scoring in bf16 (1 cyc/col): 48 one-hot matmuls with rhs spanning the
    row pair (N=392) -> PSUM lg2 [16, 392]; row m holds logits[X=m] for
    row A in cols 0:196 and row B in cols 196:392.
  - softmax without max-subtraction (|l2| <= ~6 so exp is fp32-safe):
    t2 = lg2 * (mask*wkk) [DVE], ex = Exp(t2) [ACT], then per row
    attn_un = ex_row * E with esum accumulated [DVE AMR], E =
    exp(ab*mask*wkk + bkk) host-precomputed; one reciprocal per pair;
    attn = attn_un * rcp [DVE tensor_scalar].
  - attn broadcast without HBM: SBUF->SBUF DMA gather [16,196]->[1,3136],
    then 0-stride-replication SBUF->SBUF DMA -> attnB [128, 16, 196].
  - pass B split across engines: chunks 0,1 = GpSimd tensor_mul [128,3136]
    + DVE 3D tensor_reduce (axis=X) -> osb[:, row, :]; chunk 2 = 16 DVE
    affine_mul_reduce ops (one per X).
  - outputs accumulate in SBUF [128, 8, 16] per chunk; one DMA per chunk.
"""

import sys

for _p in ("/opt/trn_rl_repo", "/root/.axon_site/_ro/trn_rl_repo"):
    if _p not in sys.path:
        sys.path.append(_p)

import ml_dtypes
import numpy as np

import concourse.bacc as bacc
import concourse.bass as bass_mod
import concourse.mybir as mybir
import concourse.tile as tile
from concourse.bass_utils import run_bass_kernel_spmd

K = 14          # patch size
C = 384         # channels
CCH = 128       # channel chunk (partitions)
NCH = C // CCH  # 3 chunks
NX = 16         # patches across W
P = K * K       # 196 pixels per patch
W2 = 2 * P      # 392 columns: a row-pair in one scoring matmul group
NCORES = 8
NROW = 8
NPAIRS = NROW // 2

FP32 = mybir.dt.float32
BF16 = mybir.dt.bfloat16

# pass B: each of the 24 (row, chunk) units does mult (DVE tensor_mul at
# 2x bf16, or GpSimd tensor_mul) into a bf16 prod tile, then 16 per-X
# tensor_scalar ops (4x_2p mode) with accum_out as the segmented reduce.
N_DVE_MULT = 8           # units whose mult runs on DVE (rest on GpSimd)


def build_nc(nrow=NROW):
    """Build the SPMD Bass program (identical on all cores)."""
    nc = bacc.Bacc("TRN2", target_bir_lowering=False, debug=False,
                   num_devices=NCORES)

    # patch-grouped shard: [c, row, X, p]
    hr = nc.dram_tensor("hr", [C, nrow, NX, P], BF16, kind="ExternalInput")
    # one-hot scorer weights: woh[c, X, m] = w[c] if m == X else 0
    woh = nc.dram_tensor("woh", [C, NX, NX], BF16, kind="ExternalInput")
    # mw2[m, pair, ri*196+p] = mask[2*pair+ri, m] * wkk[p]
    mw2 = nc.dram_tensor("mw2", [NX, NPAIRS, W2], FP32, kind="ExternalInput")
    # e196[m, r, p] = exp(ab * mask[r, m] * wkk[p] + bkk[p])
    e196 = nc.dram_tensor("e196", [NX, nrow, P], FP32, kind="ExternalInput")
    out = nc.dram_tensor("out", [C, nrow, NX], FP32, kind="ExternalOutput")
    attn_dram = nc.dram_tensor("attn_scratch", [nrow, NX, P], BF16)

    with tile.TileContext(nc) as tc:
        _emit(tc, nc, nrow, hr, woh, mw2, e196, out, attn_dram)
    nc.finalize()
    return nc


def _emit(tc, nc, nrow, hr, woh, mw2, e196, out, attn_dram):
    import contextlib

    # per-unit engine assignment (u = 3*row + chunk, 24 units):
    # mult: alternate units on GpSimd (12) and DVE (12); reduce: DVE
    # tensor_reduce, except 4 mid-schedule GpSimd-multiplied units on ACT.
    gps_mult = [u % 2 == 0 for u in range(24)]
    act_red = [u in (6, 10, 14, 16) for u in range(24)]

    ctx = contextlib.ExitStack()
    with ctx:
        singles = ctx.enter_context(tc.tile_pool(name="singles", bufs=1))
        data_pool = ctx.enter_context(tc.tile_pool(name="data", bufs=9))
        small = ctx.enter_context(tc.tile_pool(name="small", bufs=2))
        attnb_pool = ctx.enter_context(tc.tile_pool(name="attnb", bufs=3))
        prod_pool = ctx.enter_context(tc.tile_pool(name="prod", bufs=5))
        scratch_pool = ctx.enter_context(tc.tile_pool(name="scratch", bufs=1))
        psum_lg = ctx.enter_context(
            tc.tile_pool(name="psum_lg", bufs=2, space="PSUM"))

        # ---- constants (loaded once) ----
        woh_sb = singles.tile([CCH, NCH, NX, NX], BF16)
        for k in range(NCH):
            nc.sync.dma_start(out=woh_sb[:, k, :, :],
                              in_=woh[k * CCH:(k + 1) * CCH, :, :])
        mw2_sb = singles.tile([NX, NPAIRS, W2], FP32)
        nc.sync.dma_start(out=mw2_sb, in_=mw2[:, :, :])
        e196_sb = singles.tile([NX, nrow, P], FP32)
        nc.sync.dma_start(out=e196_sb, in_=e196[:, :, :])

        scratch = scratch_pool.tile([CCH, P], BF16, tag="scratch")
        scratch2 = scratch_pool.tile([CCH, P], BF16, tag="scratch2")
        osb = [singles.tile([CCH, nrow, NX], FP32, name=f"osb{k}",
                            tag=f"osb{k}") for k in range(NCH)]

        def pass_b(r, dkp, ri, attnB):
            # Emit all GpSimd mults first, then DVE mults, then reduces
            # (own-product reduces before GpSimd-dependent ones) so DVE's
            # in-order queue never blocks its own mults behind a reduce
            # that waits on GpSimd.
            prods = {}
            order = sorted(range(NCH), key=lambda k: not gps_mult[3 * r + k])
            for k in order:
                prod = prod_pool.tile([CCH, NX, P], BF16, tag="prod")
                prods[k] = prod
                if gps_mult[3 * r + k]:
                    nc.gpsimd.tensor_mul(prod, dkp[k][:, ri, :, :], attnB)
                else:
                    nc.vector.tensor_mul(prod, dkp[k][:, ri, :, :], attnB)
            for k in sorted(range(NCH),
                            key=lambda k: bool(gps_mult[3 * r + k])):
                u = 3 * r + k
                if act_red[u]:
                    for X in range(NX):
                        nc.scalar.activation(
                            scratch2, prods[k][:, X, :],
                            mybir.ActivationFunctionType.Copy,
                            accum_out=osb[k][:, r, X:X + 1])
                else:
                    nc.vector.tensor_reduce(
                        osb[k][:, r, :], prods[k], axis=mybir.AxisListType.X,
                        op=mybir.AluOpType.add)

        pending = []
        for pr in range(NPAIRS):
            # ---- load data tiles (one row pair) ----
            dk = []
            for k in range(NCH):
                t = data_pool.tile([CCH, 2, NX, P], BF16, tag="data")
                nc.sync.dma_start(
                    out=t, in_=hr[k * CCH:(k + 1) * CCH, 2 * pr:2 * pr + 2,
                                  :, :])
                dk.append(t)

            # ---- scoring: 48 one-hot matmuls (N=392), one accum group ----
            lg2 = psum_lg.tile([NX, W2], FP32, tag="lg")
            for X in range(NX):
                for k in range(NCH):
                    nc.tensor.matmul(
                        lg2[:, :],
                        woh_sb[:, k, X, :],
                        dk[k][:, :, X, :],
                        start=(X == 0 and k == 0),
                        stop=(X == NX - 1 and k == NCH - 1),
                    )

            # ---- softmax over p (no max subtraction; |l2| small) ----
            t2 = small.tile([NX, W2], FP32, tag="t2")
            nc.vector.tensor_mul(t2, lg2[:, :], mw2_sb[:, pr, :])
            ex = small.tile([NX, W2], FP32, tag="ex")
            nc.scalar.activation(ex, t2, mybir.ActivationFunctionType.Exp)
            attn_un = small.tile([NX, 2, P], FP32, tag="attn_un")
            esum = small.tile([NX, 2], FP32, tag="esum")
            for ri in range(2):
                nc.vector.affine_mul_reduce(
                    out=attn_un[:, ri, :], accum_out=esum[:, ri:ri + 1],
                    in0=ex[:, ri * P:(ri + 1) * P],
                    in1=e196_sb[:, 2 * pr + ri, :],
                    scale=1.0, bias=0.0)
            rcp = small.tile([NX, 2], FP32, tag="rcp")
            nc.vector.reciprocal(rcp, esum)
            attn = small.tile([NX, 2, P], BF16, tag="attn")
            for ri in range(2):
                nc.vector.tensor_scalar_mul(attn[:, ri, :], attn_un[:, ri, :],
                                            rcp[:, ri:ri + 1])

            # ---- per row: broadcast, then pass B of the previous row ----
            for ri in range(2):
                r = 2 * pr + ri
                attnB = attnb_pool.tile([CCH, NX, P], BF16, tag="attnB")
                nc.gpsimd.dma_start(out=attn_dram[r, :, :], in_=attn[:, ri, :])
                _src = attn_dram[r, :, :]
                _bsrc = bass_mod.AP(tensor=_src.tensor, offset=_src.offset,
                                    ap=[[0, CCH], *_src.ap])
                nc.gpsimd.dma_start(out=attnB, in_=_bsrc)
                pending.append((r, dk, ri, attnB))
                if len(pending) > 1:
                    done_r = pending[0][0]
                    pass_b(*pending.pop(0))
                    if done_r % 2 == 1:
                        pd = done_r - 1
                        for k in range(NCH):
                            nc.sync.dma_start(
                                out=out[k * CCH:(k + 1) * CCH, pd:pd + 2, :],
                                in_=osb[k][:, pd:pd + 2, :])

        for args in pending:
            pass_b(*args)

        # last pair's outputs (not flushed inside the loop)
        for k in range(NCH):
            nc.sync.dma_start(out=out[k * CCH:(k + 1) * CCH, nrow - 2:, :],
                              in_=osb[k][:, nrow - 2:, :])


_NC_CACHE = {}


def _get_nc(nrow=NROW):
    if nrow not in _NC_CACHE:
        _NC_CACHE[nrow] = build_nc(nrow)
    return _NC_CACHE[nrow]


def regroup_shard(hr_slice):
    """[384, 112, 224] -> patch-grouped bf16 [384, 8, 16, 196]."""
    c, h, w = hr_slice.shape
    g = hr_slice.reshape(c, h // K, K, w // K, K).transpose(0, 1, 3, 2, 4)
    return np.ascontiguousarray(
        g.reshape(c, h // K, w // K, P)).astype(ml_dtypes.bfloat16)


def make_in_maps(hr_feats, guidance, attn_w, attn_b, w_kk, b_kk, dropout_mask,
                 nrow=NROW):
    b = hr_feats.shape[0]
    w = np.asarray(attn_w, np.float32)[0]                      # [384]
    ab = np.float32(np.asarray(attn_b)[0])
    wkk_flat = np.asarray(w_kk, np.float32).reshape(-1)        # [196]
    bkk_flat = np.asarray(b_kk, np.float32).reshape(-1)        # [196]
    mask = np.asarray(dropout_mask).astype(np.float32)[..., 0]  # [b, H, W]

    woh = np.zeros((C, NX, NX), np.float32)
    woh[:, np.arange(NX), np.arange(NX)] = w[:, None]          # [c, X, m]
    woh = woh.astype(ml_dtypes.bfloat16)

    in_maps = []
    for core in range(NCORES):
        bi, half = divmod(core, 2)
        bi = bi % b
        hrg = regroup_shard(
            np.asarray(hr_feats[bi, :, 112 * half:112 * half + K * nrow, :],
                       np.float32))
        mrow = mask[bi, 8 * half:8 * half + nrow, :]           # [nrow, 16]
        mcol = np.ascontiguousarray(mrow.T)                    # [16(X), nrow]
        # mw2[m, pair, ri*196+p] = mask[2*pair+ri, m] * wkk[p]
        mw2 = (mcol[:, :, None] * wkk_flat[None, None, :])     # [16, nrow, 196]
        mw2 = np.ascontiguousarray(
            mw2.reshape(NX, NPAIRS, W2)).astype(np.float32)
        e196 = np.ascontiguousarray(
            np.exp(ab * mcol[:, :, None] * wkk_flat[None, None, :]
                   + bkk_flat[None, None, :])).astype(np.float32)
        in_maps.append({
            "hr": hrg, "woh": woh, "mw2": mw2, "e196": e196,
        })
    return in_maps


def kernel(hr_feats, guidance, attn_w, attn_b, w_kk, b_kk, dropout_mask,
           trace=False):
    hr_feats = np.asarray(hr_feats, np.float32)
    b, c, h, wimg = hr_feats.shape
    H = h // K
    nc = _get_nc(NROW)
    in_maps = make_in_maps(hr_feats, guidance, attn_w, attn_b, w_kk, b_kk,
                           dropout_mask)
    res = run_bass_kernel_spmd(nc, in_maps, core_ids=list(range(NCORES)),
                               trace=trace)
    full = np.empty((b, C, H, NX), np.float32)
    for core in range(NCORES):
        bi, half = divmod(core, 2)
        full[bi, :, 8 * half:8 * half + 8, :] = res.results[core]["out"]
    if trace:
        return full, res
    return full
